# revision 31
# baseline (speedup 1.0000x reference)
# Dense GAT layer (4 heads, dim 64) on Trainium2 via Bass/Tile.
#
# Math: h = x@W; e_ij = LeakyReLU(src_i + dst_j, 0.2); masked softmax over j
# with valid = adj & mask_i & mask_j; out = LN((alpha @ h) * mask_i).
#
# Key identity: softmax over j is invariant to any per-column-i scale, so
#   exp(LeakyReLU(s_i + d_j)) ~_i exp(max(0.8 s_i + d_j, 0.2 d_j))
#                             = B_j * max(q_i, R_j)
# with q_i = exp(0.8 s_i) (a row, broadcast via PE), B_j = exp(d_j),
# R_j = exp(-0.8 d_j) (per-partition scalars). One tensor_scalar
# (max, mult — 4x DVE mode) + one tensor_mul with the (mask_j-folded)
# adjacency produce the unnormalized attention u per 128-row chunk.
# Equivalent ACT form used to balance engines:
#   B_j * max(q_i, R_j) = exp(0.2 d_j) * exp(relu(s_i + d_j) * 0.8)
# (two activation ops with per-partition biases).
#
# Layout: "e^T" orientation — j (softmax axis) on partitions, i on the free
# axis, so alpha@h needs no transposes and rowsum is a ones-column in the
# alpha@h matmul rhs; the rowsum division is fused into the PSUM drain
# (tensor_scalar op0=divide). mask_i folds into the LayerNorm rstd.
# Sharding: data-parallel, 2 graphs per core across 8 cores. x and adj are
# pre-transposed on the host so all input DMAs are plain (no DMA-transpose).

import numpy as np

H, D = 4, 64
EPS = 1e-5
NCORES = 8

# Per-jc engine routing (tuning knobs).
#  m-stage: 'A' = ACT (relu+exp), 'V' = DVE tensor_scalar, 'P' = Pool
#           tensor_scalar, 'X' = A on even (g*H+h), else V.
#  u-stage (multiply by adjacency): 'V' = DVE, 'P' = Pool.
M_ROUTE = ["V", "V", "V", "V", "V", "V", "A", "A"]
T_ROUTE = ["V", "V", "V", "V", "V", "V", "V", "V"]
N_DRAIN_ACT = 6  # of the 8 per-head drains, how many go to ACT

_PROG_CACHE = {}


def _build_program(ng, n, in_dim, trivial_ln):
    import concourse.bacc as bacc
    import concourse.mybir as mybir
    import concourse.tile as tile
    from concourse.bass import ts

    f16 = mybir.dt.float16
    f32 = mybir.dt.float32
    AF = mybir.ActivationFunctionType
    OP = mybir.AluOpType

    HD = H * D
    NCH = n // 128          # node chunks
    KC = in_dim // 128      # contraction chunks for x@W
    NW = min(512, n)        # matmul moving-column chunk width
    NH = n // NW            # number of column chunks
    E = D + 1               # head block in hones (64 h cols + 1 ones col)

    nc = bacc.Bacc()

    xT = nc.dram_tensor("xT", [ng, in_dim, n], f16, kind="ExternalInput")
    adjTm = nc.dram_tensor("adjTm", [ng, n, n], f16, kind="ExternalInput")
    wc = nc.dram_tensor("wc", [128, KC * (HD + H)], f16, kind="ExternalInput")
    wsd = nc.dram_tensor("wsd", [128, KC * H], f16, kind="ExternalInput")
    mcolT = nc.dram_tensor("mcolT", [ng, 128, NCH], f32, kind="ExternalInput")
    if not trivial_ln:
        gam = nc.dram_tensor("gamma_rep", [128, HD], f16, kind="ExternalInput")
        bet = nc.dram_tensor("beta_rep", [128, HD], f16, kind="ExternalInput")
    out16 = nc.dram_tensor("out16", [ng, n, HD], f16, kind="ExternalOutput")

    from contextlib import ExitStack

    with tile.TileContext(nc) as tc, ExitStack() as ctx:
        def pool(**kw):
            return ctx.enter_context(tc.tile_pool(**kw))

        consts = pool(name="consts", bufs=1)
        xt_pool = pool(name="xt", bufs=2)
        adjt_pool = pool(name="adjt", bufs=2)
        rowp = pool(name="rowp", bufs=2)
        flat_pool = pool(name="flat", bufs=2)
        qrep_pool = pool(name="qrep", bufs=3)
        hones_pool = pool(name="hones", bufs=2 * NCH)
        small_pool = pool(name="small", bufs=8)
        ew_pool = pool(name="ew", bufs=5)
        lr_pool = pool(name="lr", bufs=3)
        u_pool = pool(name="u", bufs=2 * NCH + 2)
        osb_pool = pool(name="osb", bufs=2 * NCH)
        ln_pool = pool(name="ln", bufs=6)
        oflat_pool = pool(name="oflat", bufs=2)
        # PSUM pools (8 banks: pbig 2x2 + ph 2x1 + pav 2x1)
        ph_pool = pool(name="ph", bufs=2, space="PSUM")
        pbig_pool = pool(name="pbig", bufs=2, space="PSUM")
        pav_pool = pool(name="pav", bufs=2, space="PSUM")

        # ---- constants (wsd first: it gates the psd matmul; wc on the
        # scalar DGE so the sync DGE reaches the x chunks sooner) ----
        wsd_sb = consts.tile([128, KC * H], f16, tag="wsd")
        nc.sync.dma_start(wsd_sb[:], wsd[:])
        ones_sb = consts.tile([1, 128], f16, tag="ones")
        nc.vector.memset(ones_sb[:], 1.0)
        wc_sb = consts.tile([128, KC * (HD + H)], f16, tag="wc")
        nc.scalar.dma_start(wc_sb[:], wc[:])
        if not trivial_ln:
            gam_sb = consts.tile([128, HD], f16, tag="gam")
            nc.scalar.dma_start(gam_sb[:], gam[:])
            bet_sb = consts.tile([128, HD], f16, tag="bet")
            nc.scalar.dma_start(bet_sb[:], bet[:])
        eps_sb = consts.tile([128, 1], f32, tag="eps")
        nc.vector.memset(eps_sb[:], EPS)

        gstate = []
        nonlocal_gate = [None]
        for g in range(ng):
            # ---- input DMAs (batched; descriptors spread over the DMA
            # engines, so one dma_start per tensor is fully parallel).
            # Graph 0: x on the sync DGE, adjacency on the scalar DGE so
            # descriptor generation runs concurrently at kernel start. ----
            xt = xt_pool.tile([128, KC * n], f16, tag="xt")
            for kc in range(KC):
                for hf in range(2):
                    nc.sync.dma_start(
                        xt[:, kc * n + hf * (n // 2) : kc * n + (hf + 1) * (n // 2)],
                        xT[g, ts(kc, 128), hf * (n // 2) : (hf + 1) * (n // 2)],
                    )
            adjt = adjt_pool.tile([128, NCH * n], f16, tag="adjt")
            if g > 0 and nonlocal_gate[0] is not None:
                # gate this graph's big adjacency load behind graph g-1's
                # first attention tile so it doesn't steal DMA bandwidth
                # from the startup-critical x transfer
                gwait = small_pool.tile([1, 1], f16, tag="gwait")
                nc.gpsimd.tensor_copy(gwait[:], nonlocal_gate[0][0:1, 0:1])
            adj_q = nc.scalar if g == 0 else nc.gpsimd
            adj_q.dma_start(
                adjt[:].rearrange("p (c i) -> p c i", c=NCH),
                adjTm[g].rearrange("(c p) i -> p c i", p=128),
            )
            mcol_sb = small_pool.tile([128, NCH], f32, tag="mcol")
            nc.gpsimd.dma_start(mcol_sb[:], mcolT[g])

            # ---- src rows: psd[h, i] = (x @ Wa_src)^T ----
            psd = pbig_pool.tile([H, n], f32, tag="pbig")
            for nh in range(NH):
                for kc in range(KC):
                    nc.tensor.matmul(
                        psd[:, ts(nh, NW)],
                        wsd_sb[:, ts(kc, H)],
                        xt[:, kc * n + nh * NW : kc * n + (nh + 1) * NW],
                        start=(kc == 0),
                        stop=(kc == KC - 1),
                    )
            srow = rowp.tile([H, n], f16, tag="srow")
            nc.scalar.copy(srow[:], psd[:])
            srowx = flat_pool.tile([1, H * n], f16, tag="srowx")
            nc.sync.dma_start(
                srowx[:].rearrange("p (h w) -> p h w", h=H), srow[:]
            )

            # ---- h_ext per chunk (fp16 h + ones col) and dst collection;
            # B/R/d02 computed in two halves so head-0 elementwise can
            # start as soon as the first half of the x@W chain is done ----
            hones = []
            dsta = small_pool.tile([128, NCH * H], f32, tag="dsta")
            Bx = small_pool.tile([128, NCH * H], f32, tag="bx")
            Rx = small_pool.tile([128, NCH * H], f32, tag="rx")
            d02 = small_pool.tile([128, NCH * H], f32, tag="d02")
            for ic in range(NCH):
                ph = ph_pool.tile([128, HD + H], f32, tag="ph")
                for kc in range(KC):
                    nc.tensor.matmul(
                        ph[:],
                        xt[:, kc * n + ic * 128 : kc * n + (ic + 1) * 128],
                        wc_sb[:, ts(kc, HD + H)],
                        start=(kc == 0),
                        stop=(kc == KC - 1),
                    )
                ho = hones_pool.tile([128, H * E], f16, tag="hones")
                ho3 = ho[:].rearrange("p (h e) -> p h e", h=H)
                nc.vector.memset(ho3[:, :, D : D + 1], 1.0)
                nc.scalar.copy(
                    ho3[:, :, 0:D],
                    ph[:, 0:HD].rearrange("p (h d) -> p h d", h=H),
                )
                hones.append(ho)
                nc.vector.tensor_copy(
                    dsta[:, ts(ic, H)], ph[:, HD : HD + H]
                )
                if ic == NCH // 2 - 1 or ic == NCH - 1:
                    hs = slice(0 if ic < NCH - 1 else NCH * H // 2,
                               (ic + 1) * H)
                    nc.scalar.activation(Bx[:, hs], dsta[:, hs], AF.Exp)
                    nc.scalar.activation(
                        Rx[:, hs], dsta[:, hs], AF.Exp, scale=-0.8
                    )
                    nc.scalar.mul(d02[:, hs], dsta[:, hs], 0.2)

            # ---- heads, software-pipelined: emit srep/qrep/elementwise for
            # head h, then the alpha@h matmul chains for head h-1, so the
            # in-order PE queue never blocks the next head's broadcast ----
            o_sb = [
                osb_pool.tile([128, HD], f16, tag="osb", name=f"osb_{g}_{i}")
                for i in range(NCH)
            ]
            mv8 = ln_pool.tile([128, 2 * NCH], f32, tag="mv8", name=f"mv8_{g}")
            u_by_h = {}

            sreps = {}
            qreps = {}

            def emit_bcast(h):
                srep = pbig_pool.tile([128, n], f32, tag="pbig")
                for nh in range(NH):
                    nc.tensor.matmul(
                        srep[:, ts(nh, NW)],
                        ones_sb[:],
                        srowx[0:1, h * n + nh * NW : h * n + (nh + 1) * NW],
                        start=True,
                        stop=True,
                    )
                qrep = qrep_pool.tile([128, n], f16, tag="qrep")
                nc.scalar.activation(qrep[:], srep[:], AF.Exp, scale=0.8)
                sreps[h] = srep
                qreps[h] = qrep

            def emit_head_ew(h):
                srep = sreps.pop(h)
                qrep = qreps.pop(h)
                u_tiles = []
                for jc in range(NCH):
                    r = M_ROUTE[jc]
                    if r == "X":
                        r = "A" if (g * H + h) % 2 == 0 else "V"
                    sB = Bx[:, jc * H + h : jc * H + h + 1]
                    sR = Rx[:, jc * H + h : jc * H + h + 1]
                    if r == "A":
                        r1 = lr_pool.tile([128, n], f16, tag="r1")
                        nc.scalar.activation(
                            r1[:], srep[:], AF.Relu,
                            bias=dsta[:, jc * H + h : jc * H + h + 1],
                        )
                        m = ew_pool.tile([128, n], f16, tag="m")
                        nc.scalar.activation(
                            m[:], r1[:], AF.Exp,
                            bias=d02[:, jc * H + h : jc * H + h + 1],
                            scale=0.8,
                        )
                    else:
                        eng = nc.gpsimd if r == "P" else nc.vector
                        m = ew_pool.tile([128, n], f16, tag="m")
                        eng.tensor_scalar(
                            m[:], qrep[:], sR, sB, op0=OP.max, op1=OP.mult
                        )
                    ueng = nc.gpsimd if T_ROUTE[jc] == "P" else nc.vector
                    u = u_pool.tile([128, n], f16, tag="u")
                    ueng.tensor_mul(u[:], m[:], adjt[:, ts(jc, n)])
                    u_tiles.append(u)
                    if h == 0 and jc == 0:
                        nonlocal_gate[0] = u
                u_by_h[h] = u_tiles

            def emit_head_mm(h):
                u_tiles = u_by_h.pop(h)
                for ic in range(NCH):
                    pav = pav_pool.tile([128, E], f32, tag="pav")
                    for jc in range(NCH):
                        nc.tensor.matmul(
                            pav[:],
                            u_tiles[jc][:, ts(ic, 128)],
                            hones[jc][:, ts(h, E)],
                            start=(jc == 0),
                            stop=(jc == NCH - 1),
                        )
                    rs = ln_pool.tile([128, 1], f32, tag="rs")
                    nc.vector.reciprocal(rs[:], pav[:, D : D + 1])
                    if ic % NCH < N_DRAIN_ACT:
                        nc.scalar.mul(
                            o_sb[ic][:, ts(h, D)], pav[:, 0:D], rs[:]
                        )
                    else:
                        nc.vector.tensor_scalar(
                            o_sb[ic][:, ts(h, D)],
                            pav[:, 0:D],
                            rs[:],
                            None,
                            op0=OP.mult,
                        )
                    if h == H - 1:
                        st6 = ln_pool.tile([128, 6], f32, tag="st6")
                        nc.vector.bn_stats(st6[:], o_sb[ic][:])
                        nc.vector.bn_aggr(mv8[:, 2 * ic : 2 * ic + 2], st6[:])

            emit_bcast(0)
            for h in range(H):
                if h + 1 < H:
                    emit_bcast(h + 1)
                if h > 0:
                    # drains of head h-1 go BEFORE head h's elementwise in
                    # the ACT queue so pav PSUM tiles recycle promptly
                    emit_head_mm(h - 1)
                emit_head_ew(h)
            emit_head_mm(H - 1)
            gstate.append((mcol_sb, o_sb, mv8))

        # ---- LayerNorm apply for both graphs (deferred so the ACT
        # Sqrt-table load happens once, off the hot loop) ----
        for g in range(ng):
            mcol_sb, o_sb, mv8 = gstate[g]
            sd8 = ln_pool.tile([128, NCH], f32, tag="sd8")
            nc.scalar.activation(
                sd8[:],
                mv8[:].rearrange("p (c two) -> p c two", two=2)[:, :, 1],
                AF.Sqrt,
                bias=eps_sb[:],
            )
            rstd8 = ln_pool.tile([128, NCH], f32, tag="rstd8")
            nc.vector.reciprocal(rstd8[:], sd8[:])
            rstdc = ln_pool.tile([128, NCH], f32, tag="rstdc")
            nc.vector.tensor_mul(rstdc[:], rstd8[:], mcol_sb[:])
            # bias for the ACT LN-apply: -mu * rstdc
            nmu = ln_pool.tile([128, NCH], f32, tag="nmu")
            nc.vector.tensor_mul(
                nmu[:],
                mv8[:].rearrange("p (c two) -> p c two", two=2)[:, :, 0],
                rstdc[:],
            )
            nc.vector.tensor_scalar(
                nmu[:], nmu[:], -1.0, None, op0=OP.mult
            )
            o2g = oflat_pool.tile([128, NCH * HD], f16, tag="o2g")
            for ic in range(NCH):
                if ic % 2 == 0:
                    nc.scalar.activation(
                        o2g[:, ts(ic, HD)],
                        o_sb[ic][:],
                        AF.Identity,
                        bias=nmu[:, ic : ic + 1],
                        scale=rstdc[:, ic : ic + 1],
                    )
                else:
                    nc.vector.tensor_scalar(
                        o2g[:, ts(ic, HD)],
                        o_sb[ic][:],
                        mv8[:, 2 * ic : 2 * ic + 1],
                        rstdc[:, ic : ic + 1],
                        op0=OP.subtract,
                        op1=OP.mult,
                    )
                if not trivial_ln:
                    nc.vector.tensor_mul(
                        o2g[:, ts(ic, HD)], o2g[:, ts(ic, HD)], gam_sb[:]
                    )
                    nc.vector.tensor_add(
                        o2g[:, ts(ic, HD)], o2g[:, ts(ic, HD)], bet_sb[:]
                    )
            hc = NCH // 2
            for half in range(2):
                nc.gpsimd.dma_start(
                    out16[g, half * hc * 128 : (half + 1) * hc * 128].rearrange(
                        "(c p) f -> p c f", p=128
                    ),
                    o2g[:, half * hc * HD : (half + 1) * hc * HD].rearrange(
                        "p (c f) -> p c f", c=hc
                    ),
                )

    nc.compile()
    return nc


def _host_prep(x, adj, mask, W, a_src, a_dst, gamma, beta, ng, trivial_ln):
    """Per-core input maps (layout/dtype packing and transposes only)."""
    b, n, in_dim = x.shape
    HD = H * D
    NCH = n // 128
    KC = in_dim // 128

    # Fold attention vectors into W:  Wa[c, h] = sum_d W[c, h*D+d] * a[h, d]
    Wr = W.astype(np.float64).reshape(in_dim, H, D)
    wa_src = np.einsum("chd,hd->ch", Wr, a_src.astype(np.float64))
    wa_dst = np.einsum("chd,hd->ch", Wr, a_dst.astype(np.float64))

    wc_full = np.ascontiguousarray(
        np.concatenate(
            [W.astype(np.float16), wa_dst.astype(np.float16)], axis=1
        )
        .reshape(KC, 128, HD + H)
        .transpose(1, 0, 2)
    ).reshape(128, KC * (HD + H))
    wsd_full = np.ascontiguousarray(
        wa_src.astype(np.float16).reshape(KC, 128, H).transpose(1, 0, 2)
    ).reshape(128, KC * H)

    mask_f = (mask > 0).astype(np.float32)  # [b, n]

    in_maps = []
    for c in range(NCORES):
        gs = slice(c * ng, (c + 1) * ng)
        mg = mask_f[gs]  # [ng, n]
        mcolT = np.ascontiguousarray(
            mg.reshape(ng, NCH, 128).transpose(0, 2, 1)
        )  # [ng, 128, NCH]
        # adjTm[g, j, i] = (adj[g, i, j] != 0) * mask_j  (mask_j folded in)
        adjT = (adj[gs] != 0).transpose(0, 2, 1).astype(np.float16)
        adjT *= mg[:, :, None].astype(np.float16)
        m = {
            "xT": np.ascontiguousarray(
                x[gs].transpose(0, 2, 1)
            ).astype(np.float16),
            "adjTm": np.ascontiguousarray(adjT),
            "wc": wc_full,
            "wsd": wsd_full,
            "mcolT": mcolT.astype(np.float32),
        }
        if not trivial_ln:
            m["gamma_rep"] = np.ascontiguousarray(
                np.broadcast_to(gamma.astype(np.float16), (128, HD))
            )
            m["beta_rep"] = np.ascontiguousarray(
                np.broadcast_to(beta.astype(np.float16), (128, HD))
            )
        in_maps.append(m)
    return in_maps


def kernel(x, adj, mask, W, a_src, a_dst, gamma, beta, _trace=False):
    from concourse.bass_utils import run_bass_kernel_spmd

    b, n, in_dim = x.shape
    ng = b // NCORES
    trivial_ln = bool(np.all(gamma == 1.0) and np.all(beta == 0.0))

    key = (ng, n, in_dim, trivial_ln)
    if key not in _PROG_CACHE:
        _PROG_CACHE[key] = _build_program(*key)
    nc = _PROG_CACHE[key]

    in_maps = _host_prep(
        x, adj, mask, W, a_src, a_dst, gamma, beta, ng, trivial_ln
    )
    res = run_bass_kernel_spmd(
        nc, in_maps, core_ids=list(range(NCORES)), trace=_trace
    )
    outs = [
        res.results[c]["out16"].reshape(ng, n, H * D) for c in range(NCORES)
    ]
    full = np.concatenate(outs, axis=0).astype(np.float32)
    if _trace:
        return full, res
    return full


# revision 32
# speedup vs baseline: 1.0355x; 1.0355x over previous
# Dense GAT layer (4 heads, dim 64) on Trainium2 via Bass/Tile.
#
# Math: h = x@W; e_ij = LeakyReLU(src_i + dst_j, 0.2); masked softmax over j
# with valid = adj & mask_i & mask_j; out = LN((alpha @ h) * mask_i).
#
# Key identity: softmax over j is invariant to any per-column-i scale, so
#   exp(LeakyReLU(s_i + d_j)) ~_i exp(max(0.8 s_i + d_j, 0.2 d_j))
#                             = B_j * max(q_i, R_j)
# with q_i = exp(0.8 s_i) (a row, broadcast via PE), B_j = exp(d_j),
# R_j = exp(-0.8 d_j) (per-partition scalars). One tensor_scalar
# (max, mult — 4x DVE mode) + one tensor_mul with the (mask_j-folded)
# adjacency produce the unnormalized attention u per 128-row chunk.
# Equivalent ACT form used to balance engines:
#   B_j * max(q_i, R_j) = exp(0.2 d_j) * exp(relu(s_i + d_j) * 0.8)
# (two activation ops with per-partition biases).
#
# Layout: "e^T" orientation — j (softmax axis) on partitions, i on the free
# axis, so alpha@h needs no transposes and rowsum is a ones-column in the
# alpha@h matmul rhs; the rowsum division is fused into the PSUM drain
# (tensor_scalar op0=divide). mask_i folds into the LayerNorm rstd.
# Sharding: data-parallel, 2 graphs per core across 8 cores. x and adj are
# pre-transposed on the host so all input DMAs are plain (no DMA-transpose).

import numpy as np

H, D = 4, 64
EPS = 1e-5
NCORES = 8

# Per-jc engine routing (tuning knobs).
#  m-stage: 'A' = ACT (relu+exp), 'V' = DVE tensor_scalar, 'P' = Pool
#           tensor_scalar, 'X' = A on even (g*H+h), else V.
#  u-stage (multiply by adjacency): 'V' = DVE, 'P' = Pool.
M_ROUTE = ["V", "V", "V", "V", "V", "X", "A", "A"]
T_ROUTE = ["V", "V", "V", "V", "V", "V", "V", "V"]
N_DRAIN_ACT = 6  # of the 8 per-head drains, how many go to ACT

_PROG_CACHE = {}


def _build_program(ng, n, in_dim, trivial_ln):
    import concourse.bacc as bacc
    import concourse.mybir as mybir
    import concourse.tile as tile
    from concourse.bass import ts

    f16 = mybir.dt.float16
    f32 = mybir.dt.float32
    AF = mybir.ActivationFunctionType
    OP = mybir.AluOpType

    HD = H * D
    NCH = n // 128          # node chunks
    KC = in_dim // 128      # contraction chunks for x@W
    NW = min(512, n)        # matmul moving-column chunk width
    NH = n // NW            # number of column chunks
    E = D + 1               # head block in hones (64 h cols + 1 ones col)

    nc = bacc.Bacc()

    xT = nc.dram_tensor("xT", [ng, in_dim, n], f16, kind="ExternalInput")
    adjTm = nc.dram_tensor("adjTm", [ng, n, n], f16, kind="ExternalInput")
    wc = nc.dram_tensor("wc", [128, KC * (HD + H)], f16, kind="ExternalInput")
    wsd = nc.dram_tensor("wsd", [128, KC * H], f16, kind="ExternalInput")
    mcolT = nc.dram_tensor("mcolT", [ng, 128, NCH], f32, kind="ExternalInput")
    if not trivial_ln:
        gam = nc.dram_tensor("gamma_rep", [128, HD], f16, kind="ExternalInput")
        bet = nc.dram_tensor("beta_rep", [128, HD], f16, kind="ExternalInput")
    out16 = nc.dram_tensor("out16", [ng, n, HD], f16, kind="ExternalOutput")

    from contextlib import ExitStack

    with tile.TileContext(nc) as tc, ExitStack() as ctx:
        def pool(**kw):
            return ctx.enter_context(tc.tile_pool(**kw))

        consts = pool(name="consts", bufs=1)
        xt_pool = pool(name="xt", bufs=2)
        adjt_pool = pool(name="adjt", bufs=2)
        rowp = pool(name="rowp", bufs=2)
        flat_pool = pool(name="flat", bufs=2)
        qrep_pool = pool(name="qrep", bufs=3)
        hones_pool = pool(name="hones", bufs=2 * NCH)
        small_pool = pool(name="small", bufs=8)
        ew_pool = pool(name="ew", bufs=5)
        lr_pool = pool(name="lr", bufs=3)
        u_pool = pool(name="u", bufs=2 * NCH + 2)
        osb_pool = pool(name="osb", bufs=2 * NCH)
        ln_pool = pool(name="ln", bufs=6)
        oflat_pool = pool(name="oflat", bufs=2)
        # PSUM pools (8 banks: pbig 2x2 + ph 2x1 + pav 2x1)
        ph_pool = pool(name="ph", bufs=2, space="PSUM")
        pbig_pool = pool(name="pbig", bufs=2, space="PSUM")
        pav_pool = pool(name="pav", bufs=2, space="PSUM")

        # ---- constants (wsd first: it gates the psd matmul; wc on the
        # scalar DGE so the sync DGE reaches the x chunks sooner) ----
        wsd_sb = consts.tile([128, KC * H], f16, tag="wsd")
        nc.sync.dma_start(wsd_sb[:], wsd[:])
        ones_sb = consts.tile([1, 128], f16, tag="ones")
        nc.vector.memset(ones_sb[:], 1.0)
        wc_sb = consts.tile([128, KC * (HD + H)], f16, tag="wc")
        nc.scalar.dma_start(wc_sb[:], wc[:])
        if not trivial_ln:
            gam_sb = consts.tile([128, HD], f16, tag="gam")
            nc.scalar.dma_start(gam_sb[:], gam[:])
            bet_sb = consts.tile([128, HD], f16, tag="bet")
            nc.scalar.dma_start(bet_sb[:], bet[:])
        eps_sb = consts.tile([128, 1], f32, tag="eps")
        nc.vector.memset(eps_sb[:], EPS)

        gstate = []
        for g in range(ng):
            # ---- input DMAs (batched; descriptors spread over the DMA
            # engines, so one dma_start per tensor is fully parallel).
            # Graph 0: x on the sync DGE, adjacency on the scalar DGE so
            # descriptor generation runs concurrently at kernel start. ----
            xt = xt_pool.tile([128, KC * n], f16, tag="xt")
            for kc in range(KC):
                nc.sync.dma_start(
                    xt[:, ts(kc, n)], xT[g, ts(kc, 128)]
                )
            adjt = adjt_pool.tile([128, NCH * n], f16, tag="adjt")
            adj_q = nc.scalar if g == 0 else nc.gpsimd
            adj_q.dma_start(
                adjt[:].rearrange("p (c i) -> p c i", c=NCH),
                adjTm[g].rearrange("(c p) i -> p c i", p=128),
            )
            mcol_sb = small_pool.tile([128, NCH], f32, tag="mcol")
            nc.gpsimd.dma_start(mcol_sb[:], mcolT[g])

            # ---- src rows: psd[h, i] = (x @ Wa_src)^T ----
            psd = pbig_pool.tile([H, n], f32, tag="pbig")
            for nh in range(NH):
                for kc in range(KC):
                    nc.tensor.matmul(
                        psd[:, ts(nh, NW)],
                        wsd_sb[:, ts(kc, H)],
                        xt[:, kc * n + nh * NW : kc * n + (nh + 1) * NW],
                        start=(kc == 0),
                        stop=(kc == KC - 1),
                    )
            srow = rowp.tile([H, n], f16, tag="srow")
            nc.scalar.copy(srow[:], psd[:])
            srowx = flat_pool.tile([1, H * n], f16, tag="srowx")
            nc.sync.dma_start(
                srowx[:].rearrange("p (h w) -> p h w", h=H), srow[:]
            )

            # ---- h_ext per chunk (fp16 h + ones col) and dst collection;
            # B/R/d02 computed in two halves so head-0 elementwise can
            # start as soon as the first half of the x@W chain is done ----
            hones = []
            dsta = small_pool.tile([128, NCH * H], f32, tag="dsta")
            Bx = small_pool.tile([128, NCH * H], f32, tag="bx")
            Rx = small_pool.tile([128, NCH * H], f32, tag="rx")
            d02 = small_pool.tile([128, NCH * H], f32, tag="d02")
            for ic in range(NCH):
                ph = ph_pool.tile([128, HD + H], f32, tag="ph")
                for kc in range(KC):
                    nc.tensor.matmul(
                        ph[:],
                        xt[:, kc * n + ic * 128 : kc * n + (ic + 1) * 128],
                        wc_sb[:, ts(kc, HD + H)],
                        start=(kc == 0),
                        stop=(kc == KC - 1),
                    )
                ho = hones_pool.tile([128, H * E], f16, tag="hones")
                ho3 = ho[:].rearrange("p (h e) -> p h e", h=H)
                nc.gpsimd.memset(ho3[:, :, D : D + 1], 1.0)
                nc.scalar.copy(
                    ho3[:, :, 0:D],
                    ph[:, 0:HD].rearrange("p (h d) -> p h d", h=H),
                )
                hones.append(ho)
                nc.vector.tensor_copy(
                    dsta[:, ts(ic, H)], ph[:, HD : HD + H]
                )  # PSUM read: stays on DVE (Pool has no PSUM port)
                if ic == NCH // 2 - 1 or ic == NCH - 1:
                    hs = slice(0 if ic < NCH - 1 else NCH * H // 2,
                               (ic + 1) * H)
                    nc.scalar.activation(Bx[:, hs], dsta[:, hs], AF.Exp)
                    nc.scalar.activation(
                        Rx[:, hs], dsta[:, hs], AF.Exp, scale=-0.8
                    )
                    nc.scalar.mul(d02[:, hs], dsta[:, hs], 0.2)

            # ---- heads, software-pipelined: emit srep/qrep/elementwise for
            # head h, then the alpha@h matmul chains for head h-1, so the
            # in-order PE queue never blocks the next head's broadcast ----
            o_sb = [
                osb_pool.tile([128, HD], f16, tag="osb", name=f"osb_{g}_{i}")
                for i in range(NCH)
            ]
            mv8 = ln_pool.tile([128, 2 * NCH], f32, tag="mv8", name=f"mv8_{g}")
            u_by_h = {}

            sreps = {}
            qreps = {}

            def emit_bcast(h):
                srep = pbig_pool.tile([128, n], f32, tag="pbig")
                for nh in range(NH):
                    nc.tensor.matmul(
                        srep[:, ts(nh, NW)],
                        ones_sb[:],
                        srowx[0:1, h * n + nh * NW : h * n + (nh + 1) * NW],
                        start=True,
                        stop=True,
                    )
                qrep = qrep_pool.tile([128, n], f16, tag="qrep")
                nc.scalar.activation(qrep[:], srep[:], AF.Exp, scale=0.8)
                sreps[h] = srep
                qreps[h] = qrep

            def emit_head_ew(h):
                srep = sreps.pop(h)
                qrep = qreps.pop(h)
                u_tiles = []
                for jc in range(NCH):
                    r = M_ROUTE[jc]
                    if r == "X":
                        r = "A" if (g * H + h) % 4 == 0 else "V"
                    sB = Bx[:, jc * H + h : jc * H + h + 1]
                    sR = Rx[:, jc * H + h : jc * H + h + 1]
                    if r == "A":
                        r1 = lr_pool.tile([128, n], f16, tag="r1")
                        nc.scalar.activation(
                            r1[:], srep[:], AF.Relu,
                            bias=dsta[:, jc * H + h : jc * H + h + 1],
                        )
                        m = ew_pool.tile([128, n], f16, tag="m")
                        nc.scalar.activation(
                            m[:], r1[:], AF.Exp,
                            bias=d02[:, jc * H + h : jc * H + h + 1],
                            scale=0.8,
                        )
                    else:
                        eng = nc.gpsimd if r == "P" else nc.vector
                        m = ew_pool.tile([128, n], f16, tag="m")
                        eng.tensor_scalar(
                            m[:], qrep[:], sR, sB, op0=OP.max, op1=OP.mult
                        )
                    ueng = nc.gpsimd if T_ROUTE[jc] == "P" else nc.vector
                    u = u_pool.tile([128, n], f16, tag="u")
                    ueng.tensor_mul(u[:], m[:], adjt[:, ts(jc, n)])
                    u_tiles.append(u)
                u_by_h[h] = u_tiles

            def emit_head_mm(h):
                u_tiles = u_by_h.pop(h)
                for ic in range(NCH):
                    pav = pav_pool.tile([128, E], f32, tag="pav")
                    for jc in range(NCH):
                        nc.tensor.matmul(
                            pav[:],
                            u_tiles[jc][:, ts(ic, 128)],
                            hones[jc][:, ts(h, E)],
                            start=(jc == 0),
                            stop=(jc == NCH - 1),
                        )
                    rs = ln_pool.tile([128, 1], f32, tag="rs")
                    nc.vector.reciprocal(rs[:], pav[:, D : D + 1])
                    if ic % NCH < N_DRAIN_ACT:
                        nc.scalar.mul(
                            o_sb[ic][:, ts(h, D)], pav[:, 0:D], rs[:]
                        )
                    else:
                        nc.vector.tensor_scalar(
                            o_sb[ic][:, ts(h, D)],
                            pav[:, 0:D],
                            rs[:],
                            None,
                            op0=OP.mult,
                        )
                    if h == H - 1:
                        st6 = ln_pool.tile([128, 6], f32, tag="st6")
                        nc.vector.bn_stats(st6[:], o_sb[ic][:])
                        nc.vector.bn_aggr(mv8[:, 2 * ic : 2 * ic + 2], st6[:])

            emit_bcast(0)
            for h in range(H):
                if h + 1 < H:
                    emit_bcast(h + 1)
                if h > 0:
                    # drains of head h-1 go BEFORE head h's elementwise in
                    # the ACT queue so pav PSUM tiles recycle promptly
                    emit_head_mm(h - 1)
                emit_head_ew(h)
            emit_head_mm(H - 1)
            gstate.append((mcol_sb, o_sb, mv8))

        # ---- LayerNorm apply for both graphs (deferred so the ACT
        # Sqrt-table load happens once, off the hot loop) ----
        for g in range(ng):
            mcol_sb, o_sb, mv8 = gstate[g]
            sd8 = ln_pool.tile([128, NCH], f32, tag="sd8")
            nc.scalar.activation(
                sd8[:],
                mv8[:].rearrange("p (c two) -> p c two", two=2)[:, :, 1],
                AF.Sqrt,
                bias=eps_sb[:],
            )
            rstd8 = ln_pool.tile([128, NCH], f32, tag="rstd8")
            nc.vector.reciprocal(rstd8[:], sd8[:])
            rstdc = ln_pool.tile([128, NCH], f32, tag="rstdc")
            nc.vector.tensor_mul(rstdc[:], rstd8[:], mcol_sb[:])
            # bias for the ACT LN-apply: -mu * rstdc
            nmu = ln_pool.tile([128, NCH], f32, tag="nmu")
            nc.vector.tensor_mul(
                nmu[:],
                mv8[:].rearrange("p (c two) -> p c two", two=2)[:, :, 0],
                rstdc[:],
            )
            nc.vector.tensor_scalar(
                nmu[:], nmu[:], -1.0, None, op0=OP.mult
            )
            o2g = oflat_pool.tile([128, NCH * HD], f16, tag="o2g")
            for ic in range(NCH):
                if ic % 2 == 0:
                    nc.scalar.activation(
                        o2g[:, ts(ic, HD)],
                        o_sb[ic][:],
                        AF.Identity,
                        bias=nmu[:, ic : ic + 1],
                        scale=rstdc[:, ic : ic + 1],
                    )
                else:
                    nc.vector.tensor_scalar(
                        o2g[:, ts(ic, HD)],
                        o_sb[ic][:],
                        mv8[:, 2 * ic : 2 * ic + 1],
                        rstdc[:, ic : ic + 1],
                        op0=OP.subtract,
                        op1=OP.mult,
                    )
                if not trivial_ln:
                    nc.vector.tensor_mul(
                        o2g[:, ts(ic, HD)], o2g[:, ts(ic, HD)], gam_sb[:]
                    )
                    nc.vector.tensor_add(
                        o2g[:, ts(ic, HD)], o2g[:, ts(ic, HD)], bet_sb[:]
                    )
            hc = NCH // 2
            for half in range(2):
                nc.gpsimd.dma_start(
                    out16[g, half * hc * 128 : (half + 1) * hc * 128].rearrange(
                        "(c p) f -> p c f", p=128
                    ),
                    o2g[:, half * hc * HD : (half + 1) * hc * HD].rearrange(
                        "p (c f) -> p c f", c=hc
                    ),
                )

    nc.compile()
    return nc


def _host_prep(x, adj, mask, W, a_src, a_dst, gamma, beta, ng, trivial_ln):
    """Per-core input maps (layout/dtype packing and transposes only)."""
    b, n, in_dim = x.shape
    HD = H * D
    NCH = n // 128
    KC = in_dim // 128

    # Fold attention vectors into W:  Wa[c, h] = sum_d W[c, h*D+d] * a[h, d]
    Wr = W.astype(np.float64).reshape(in_dim, H, D)
    wa_src = np.einsum("chd,hd->ch", Wr, a_src.astype(np.float64))
    wa_dst = np.einsum("chd,hd->ch", Wr, a_dst.astype(np.float64))

    wc_full = np.ascontiguousarray(
        np.concatenate(
            [W.astype(np.float16), wa_dst.astype(np.float16)], axis=1
        )
        .reshape(KC, 128, HD + H)
        .transpose(1, 0, 2)
    ).reshape(128, KC * (HD + H))
    wsd_full = np.ascontiguousarray(
        wa_src.astype(np.float16).reshape(KC, 128, H).transpose(1, 0, 2)
    ).reshape(128, KC * H)

    mask_f = (mask > 0).astype(np.float32)  # [b, n]

    in_maps = []
    for c in range(NCORES):
        gs = slice(c * ng, (c + 1) * ng)
        mg = mask_f[gs]  # [ng, n]
        mcolT = np.ascontiguousarray(
            mg.reshape(ng, NCH, 128).transpose(0, 2, 1)
        )  # [ng, 128, NCH]
        # adjTm[g, j, i] = (adj[g, i, j] != 0) * mask_j  (mask_j folded in)
        adjT = (adj[gs] != 0).transpose(0, 2, 1).astype(np.float16)
        adjT *= mg[:, :, None].astype(np.float16)
        m = {
            "xT": np.ascontiguousarray(
                x[gs].transpose(0, 2, 1)
            ).astype(np.float16),
            "adjTm": np.ascontiguousarray(adjT),
            "wc": wc_full,
            "wsd": wsd_full,
            "mcolT": mcolT.astype(np.float32),
        }
        if not trivial_ln:
            m["gamma_rep"] = np.ascontiguousarray(
                np.broadcast_to(gamma.astype(np.float16), (128, HD))
            )
            m["beta_rep"] = np.ascontiguousarray(
                np.broadcast_to(beta.astype(np.float16), (128, HD))
            )
        in_maps.append(m)
    return in_maps


def kernel(x, adj, mask, W, a_src, a_dst, gamma, beta, _trace=False):
    from concourse.bass_utils import run_bass_kernel_spmd

    b, n, in_dim = x.shape
    ng = b // NCORES
    trivial_ln = bool(np.all(gamma == 1.0) and np.all(beta == 0.0))

    key = (ng, n, in_dim, trivial_ln)
    if key not in _PROG_CACHE:
        _PROG_CACHE[key] = _build_program(*key)
    nc = _PROG_CACHE[key]

    in_maps = _host_prep(
        x, adj, mask, W, a_src, a_dst, gamma, beta, ng, trivial_ln
    )
    res = run_bass_kernel_spmd(
        nc, in_maps, core_ids=list(range(NCORES)), trace=_trace
    )
    outs = [
        res.results[c]["out16"].reshape(ng, n, H * D) for c in range(NCORES)
    ]
    full = np.concatenate(outs, axis=0).astype(np.float32)
    if _trace:
        return full, res
    return full
